# revision 5
# baseline (speedup 1.0000x reference)
"""Trainium2 Bass kernel for nn_ActorCritic (8-core SPMD, tensor-parallel heads).

Strategy (memory-bound regime, ~1.29 GB of f32 weights dominate):
  - Host pre-transposes/pre-tiles every weight matrix into a per-core flat
    stream of [128,128] lhsT blocks, in the exact order the TensorEngine
    consumes them.  Device DMA is pure sequential streaming of ~4 MB chunks.
  - Each core owns a 1/8 output-dim shard of every head's first layer and the
    matching input-dim shard of the second layer, so the only cross-core
    traffic is one fused AllReduce of second-layer partial sums per head
    group (mon/end/critic fused; card separate, overlapped with compute).
  - Weights are the stationary matmul operand (LDWEIGHTS ~107ns / 64KB block
    => ~600 GB/s PE consumption vs ~360 GB/s HBM DMA: DMA-bound as intended).
  - The tiny finale (third-layer heads, layernorm'd pooling trunk, softmax)
    is computed redundantly on every core.
"""

import numpy as np

# ---------------------------------------------------------------- dimensions
D = 512
ENC = 64
MAX_HAND = 10
HAND, DRAW, DISC = 7, 20, 15
D12, D13 = 12 * D, 13 * D            # 6144, 6656
OTH = 48 + 64 + 8                    # 120
NCORES = 8
CSH = D13 // NCORES                  # 832  card out-shard
CSHP = 896                           # padded to 7*128
PSH = D12 // NCORES                  # 768  = 6*128 p-head out-shard
KG = D12 // 128                      # 48 k-tiles of g
KH = 4                               # k-tiles of x_hand part
CJ = CSHP // 128                     # 7
PJ = PSH // 128                      # 6
CK2 = CSHP // 128                    # 7 k-tiles for card W1
LN_EPS = 1e-5

# stream block counts (order: otherW0, otherW1, mon W0/W1, end, critic, cardW0, cardW1)
NB_OW0 = 12
NB_OW1 = 144
NB_PW0 = PJ * KG                     # 288
NB_PW1 = KG * PJ                     # 288
NB_CW0 = CJ * 52                     # 364   (52 = 48 g-tiles + 4 hand-tiles)
NB_CW1 = 52 * CK2                    # 364
NB = NB_OW0 + NB_OW1 + 3 * (NB_PW0 + NB_PW1) + NB_CW0 + NB_CW1   # 2612

CB = 64                              # blocks per DMA chunk (4 MB)

F32 = np.float32
WDT = "f32"            # weight-stream dtype: "f32" or "bf16"


def _np_wdt():
    if WDT == "bf16":
        import ml_dtypes
        return ml_dtypes.bfloat16
    return np.float32


# ---------------------------------------------------------------- host packing
def _blocks(Wc):
    """Wc [J*128 out, K*128 in] -> [J*K,128,128] lhsT blocks, j-outer k-inner.
    block(j,k)[p,m] = Wc[128j+m, 128k+p]  (K on partitions, out on free)."""
    J, K = Wc.shape[0] // 128, Wc.shape[1] // 128
    return np.ascontiguousarray(
        Wc.reshape(J, 128, K, 128).transpose(0, 2, 3, 1)).reshape(J * K, 128, 128)


def _pad2(a, r, c):
    out = np.zeros((r, c), F32)
    out[: a.shape[0], : a.shape[1]] = a
    return out


SMALLS_LAYOUT = [
    ("emb_WT", 512), ("handT", 10), ("drawT", 20), ("discT", 15),
    ("emb_b2d", 4), ("oin", 1), ("ob0", 12), ("ob1", 12),
    ("card_b0T", 7), ("card_b1bc", 520), ("p_b0", 18), ("p_b1", 144),
    ("cardWoutT", 104), ("pWoutT", 144), ("lng", 48), ("lnb", 48),
    ("addvec", 22), ("cbout", 1),
]
SM_OFF = {}
_off = 0
for _n, _w in SMALLS_LAYOUT:
    SM_OFF[_n] = (_off, _w)
    _off += _w
SM_W = _off

PHEADS = ["mon", "end", "critic"]


def _col2d(v, ncol):
    """v [ncol*128] -> [128, ncol] with col j = v[128j:128j+128]."""
    return np.ascontiguousarray(v.reshape(ncol, 128).T)


def pack_smalls(c, hand, draw_pile, disc_pile, character, monster, energy,
                valid_action_mask, p):
    sm = np.zeros((128, SM_W), F32)

    def put(name, arr, rows=128):
        o, w = SM_OFF[name]
        sm[:rows, o:o + w] = arr
    put("emb_WT", p["emb_W"].T, rows=64)
    handp = np.concatenate(
        [hand, np.broadcast_to(p["card_pad"], (MAX_HAND - hand.shape[0], ENC))], 0)
    put("handT", handp.T, rows=64)
    put("drawT", draw_pile.T, rows=64)
    put("discT", disc_pile.T, rows=64)
    put("emb_b2d", _col2d(p["emb_b"], 4))
    oin = np.zeros((128, 1), F32)
    oin[:OTH, 0] = np.concatenate([character, monster, energy])
    put("oin", oin)
    put("ob0", _col2d(p["other_b0"], 12))
    put("ob1", _col2d(p["other_b1"], 12))
    b0c = np.zeros(CSHP, F32)
    b0c[:CSH] = p["card_b0"][c * CSH:(c + 1) * CSH]
    put("card_b0T", _col2d(b0c, CJ))
    put("card_b1bc", np.repeat(_col2d(p["card_b1"], 52), MAX_HAND, axis=1))
    pb0 = np.concatenate(
        [_col2d(p[h + "_b0"][c * PSH:(c + 1) * PSH], PJ) for h in PHEADS], 1)
    put("p_b0", pb0)
    put("p_b1", np.concatenate([_col2d(p[h + "_b1"], KG) for h in PHEADS], 1))
    put("cardWoutT",
        np.ascontiguousarray(p["card_Wout"].T.reshape(52, 128, 2)
                             .transpose(1, 0, 2)).reshape(128, 104))
    put("pWoutT", np.concatenate([_col2d(p[h + "_Wout"][0], KG) for h in PHEADS], 1))
    put("lng", _col2d(p["ln_g"], KG))
    put("lnb", _col2d(p["ln_b"], KG))
    av = np.concatenate([np.full(10, p["card_bout"][0], F32),
                         np.full(10, p["card_bout"][1], F32),
                         p["mon_bout"].astype(F32), p["end_bout"].astype(F32)])
    av = av + np.where(valid_action_mask == 0, F32(-1e30), F32(0.0))
    put("addvec", av[None, :], rows=1)
    put("cbout", p["critic_bout"].astype(F32)[None, :], rows=1)
    return sm


def pack_wstream(c, p):
    segs = []
    segs.append(_blocks(_pad2(p["other_W0"], 12 * 128, 128)))          # in 120->128
    segs.append(_blocks(p["other_W1"]))
    for h in PHEADS:
        segs.append(_blocks(p[h + "_W0"][c * PSH:(c + 1) * PSH, :]))
        segs.append(_blocks(p[h + "_W1"][:, c * PSH:(c + 1) * PSH]))
    cw0 = np.zeros((CSHP, D13), F32)
    cw0[:CSH] = p["card_W0"][c * CSH:(c + 1) * CSH, :]
    segs.append(_blocks(cw0))
    cw1 = np.zeros((D13, CSHP), F32)
    cw1[:, :CSH] = p["card_W1"][:, c * CSH:(c + 1) * CSH]
    segs.append(_blocks(cw1))
    B = np.concatenate(segs, 0)
    assert B.shape == (NB, 128, 128), B.shape
    img = np.ascontiguousarray(B.transpose(1, 0, 2)).reshape(128, NB * 128)
    return img.astype(_np_wdt())


# ---------------------------------------------------------------- graph build
def build_graph():
    import concourse.bass as bass
    import concourse.tile as tile
    from concourse import bacc, mybir

    f32 = mybir.dt.float32
    wdt = mybir.dt.bfloat16 if WDT == "bf16" else mybir.dt.float32
    AX = mybir.AxisListType.X
    ADD = mybir.AluOpType.add
    SUB = mybir.AluOpType.subtract
    MUL = mybir.AluOpType.mult
    MAX = mybir.AluOpType.max

    nc = bacc.Bacc("TRN2", target_bir_lowering=False, debug=False,
                   num_devices=NCORES)
    wstream = nc.dram_tensor("wstream", [128, NB * 128], wdt,
                             kind="ExternalInput").ap()
    smalls = nc.dram_tensor("smalls", [128, SM_W], f32,
                            kind="ExternalInput").ap()
    out_ext = nc.dram_tensor("out", [1, 23], f32, kind="ExternalOutput").ap()

    with tile.TileContext(nc) as tc:
        with (tc.tile_pool(name="persist", bufs=1) as pers,
              tc.tile_pool(name="wring", bufs=4) as wring,
              tc.tile_pool(name="temps", bufs=3) as temps,
              tc.tile_pool(name="acc1", bufs=2, space="PSUM") as pac1,
              tc.tile_pool(name="acc10", bufs=2, space="PSUM") as pac10,
              tc.tile_pool(name="pmisc", bufs=1, space="PSUM") as pmisc,
              tc.tile_pool(name="dram", bufs=1, space="DRAM") as dpool):

            sm = pers.tile([128, SM_W], f32)
            nc.sync.dma_start(out=sm, in_=smalls)

            def S(name, rows=128):
                o, w = SM_OFF[name]
                return sm[:rows, o:o + w]

            def Sc(name, j, w=1, rows=128):
                o, _ = SM_OFF[name]
                return sm[:rows, o + j * w:o + (j + 1) * w]

            # persistent SBUF buffers
            xhT = pers.tile([128, 40], f32)
            xhT_w = pers.tile([128, 40], wdt)          # x_hand.T tiles [128,10]*4
            graw = pers.tile([128, KG], f32)
            gsq = pers.tile([128, KG], f32)
            gn = pers.tile([128, KG], wdt)
            oh0T = pers.tile([128, 12], wdt)
            ph0T = pers.tile([128, 18], wdt)
            h0cT = pers.tile([128, 70], wdt)
            p_part = pers.tile([128, 144], f32)
            card_part = pers.tile([128, 520], f32)
            p_red = pers.tile([128, 144], f32)
            ph1 = pers.tile([128, 144], f32)
            c_red = pers.tile([128, 520], f32)
            ch1 = pers.tile([128, 520], f32)
            ones128 = pers.tile([128, 1], f32)
            ones1 = pers.tile([1, 128], f32)
            eps_sb = pers.tile([1, 1], f32)
            stat = pers.tile([1, 8], f32)            # sg, sq, m, esq, var, std, rstd
            mean_bc = pers.tile([128, 1], f32)
            rstd_bc = pers.tile([128, 1], f32)
            logit = pers.tile([1, 22], f32)
            lg2 = pers.tile([1, 22], f32)
            ex = pers.tile([1, 22], f32)
            out_sb = pers.tile([1, 23], f32)

            nc.vector.memset(ones128, 1.0)
            nc.vector.memset(ones1, 1.0)
            nc.vector.memset(eps_sb, LN_EPS)

            # ---------------- stage A: embeddings + pools -------------------
            # hand -> xhT tiles + pools (g cols 0-11)
            for t in range(4):
                ps = pac10.tile([128, 20], f32, tag="emb")
                nc.tensor.matmul(ps[:, :10], S("emb_WT", 64)[:, t * 128:(t + 1) * 128],
                                 S("handT", 64), start=True, stop=True)
                nc.vector.tensor_scalar_add(out=xhT[:, t * 10:(t + 1) * 10],
                                            in0=ps[:, :10], scalar1=Sc("emb_b2d", t))
                sl = xhT[:, t * 10:(t + 1) * 10]
                nc.vector.reduce_sum(out=graw[:, 4 + t:5 + t], in_=sl, axis=AX)
                nc.vector.tensor_scalar_mul(out=graw[:, t:t + 1],
                                            in0=graw[:, 4 + t:5 + t], scalar1=0.1)
                nc.vector.reduce_max(out=graw[:, 8 + t:9 + t], in_=sl, axis=AX)
            nc.vector.tensor_copy(out=xhT_w, in_=xhT)
            # draw (g cols 12-23), disc (24-35)
            for nm, n, base, scale in (("drawT", DRAW, 12, 1.0 / DRAW),
                                       ("discT", DISC, 24, 1.0 / DISC)):
                for t in range(4):
                    ps = pac10.tile([128, 20], f32, tag="emb")
                    nc.tensor.matmul(ps[:, :n],
                                     S("emb_WT", 64)[:, t * 128:(t + 1) * 128],
                                     S(nm, 64), start=True, stop=True)
                    xe = temps.tile([128, 20], f32, tag="xe")
                    nc.vector.tensor_scalar_add(out=xe[:, :n], in0=ps[:, :n],
                                                scalar1=Sc("emb_b2d", t))
                    nc.vector.reduce_sum(out=graw[:, base + 4 + t:base + 5 + t],
                                         in_=xe[:, :n], axis=AX)
                    nc.vector.tensor_scalar_mul(out=graw[:, base + t:base + t + 1],
                                                in0=graw[:, base + 4 + t:base + 5 + t],
                                                scalar1=scale)
                    nc.vector.reduce_max(out=graw[:, base + 8 + t:base + 9 + t],
                                         in_=xe[:, :n], axis=AX)

            # ---------------- weight stream ---------------------------------
            state = {"b": 0, "cur": None}

            def next_block():
                cb, off = divmod(state["b"], CB)
                if off == 0:
                    nb = min(CB, NB - cb * CB)
                    state["cur"] = wring.tile([128, nb * 128], wdt, tag="wchunk",
                                              name="wchunk%d" % cb)
                    nc.sync.dma_start(
                        out=state["cur"],
                        in_=wstream[:, cb * CB * 128: cb * CB * 128 + nb * 128])
                state["b"] += 1
                return state["cur"][:, off * 128:(off + 1) * 128]

            # other MLP layer 0: 12 blocks, rhs = oin (cast to stream dtype)
            oin_w = pers.tile([128, 1], wdt)
            nc.vector.tensor_copy(out=oin_w, in_=S("oin"))
            for j in range(12):
                ps = pac1.tile([128, 1], f32, tag="a1")
                nc.tensor.matmul(ps, next_block(), oin_w, start=True, stop=True)
                nc.vector.tensor_scalar(out=oh0T[:, j:j + 1], in0=ps,
                                        scalar1=Sc("ob0", j), scalar2=0.0,
                                        op0=ADD, op1=MAX)
            # other MLP layer 1: 12x12 -> graw cols 36-47
            for j in range(12):
                ps = pac1.tile([128, 1], f32, tag="a1")
                for k in range(12):
                    nc.tensor.matmul(ps, next_block(), oh0T[:, k:k + 1],
                                     start=(k == 0), stop=(k == 11))
                nc.vector.tensor_scalar(out=graw[:, 36 + j:37 + j], in0=ps,
                                        scalar1=Sc("ob1", j), scalar2=0.0,
                                        op0=ADD, op1=MAX)

            # ---------------- layernorm over g ------------------------------
            nc.vector.tensor_mul(gsq, graw, graw)
            pcs = pmisc.tile([1, 48], f32, tag="mln")
            nc.tensor.matmul(pcs, ones128, graw, start=True, stop=True)
            nc.vector.reduce_sum(out=stat[:, 0:1], in_=pcs, axis=AX)
            pcs2 = pmisc.tile([1, 48], f32, tag="mln")
            nc.tensor.matmul(pcs2, ones128, gsq, start=True, stop=True)
            nc.vector.reduce_sum(out=stat[:, 1:2], in_=pcs2, axis=AX)
            nc.vector.tensor_scalar_mul(out=stat[:, 2:3], in0=stat[:, 0:1],
                                        scalar1=1.0 / D12)        # mean
            nc.vector.tensor_scalar_mul(out=stat[:, 3:4], in0=stat[:, 1:2],
                                        scalar1=1.0 / D12)        # E[g^2]
            nc.vector.tensor_mul(stat[:, 4:5], stat[:, 2:3], stat[:, 2:3])
            nc.vector.tensor_sub(stat[:, 5:6], stat[:, 3:4], stat[:, 4:5])  # var
            nc.scalar.activation(out=stat[:, 6:7], in_=stat[:, 5:6],
                                 func=mybir.ActivationFunctionType.Sqrt,
                                 bias=eps_sb, scale=1.0)
            nc.vector.reciprocal(out=stat[:, 7:8], in_=stat[:, 6:7])  # rstd
            psm = pmisc.tile([128, 1], f32, tag="mln")
            nc.tensor.matmul(psm, ones1, stat[:, 2:3], start=True, stop=True)
            nc.vector.tensor_copy(out=mean_bc, in_=psm)
            psr = pmisc.tile([128, 1], f32, tag="mln")
            nc.tensor.matmul(psr, ones1, stat[:, 7:8], start=True, stop=True)
            nc.vector.tensor_copy(out=rstd_bc, in_=psr)
            tmp48 = temps.tile([128, KG], f32, tag="t48")
            nc.vector.tensor_scalar(out=tmp48, in0=graw, scalar1=mean_bc,
                                    scalar2=rstd_bc, op0=SUB, op1=MUL)
            nc.vector.tensor_mul(tmp48, tmp48, S("lng"))
            nc.vector.tensor_add(gn, tmp48, S("lnb"))

            # ---------------- mon/end/critic heads (shards) -----------------
            for h in range(3):
                for j in range(PJ):
                    ps = pac1.tile([128, 1], f32, tag="a1")
                    for k in range(KG):
                        nc.tensor.matmul(ps, next_block(), gn[:, k:k + 1],
                                         start=(k == 0), stop=(k == KG - 1))
                    nc.vector.tensor_scalar(out=ph0T[:, 6 * h + j:6 * h + j + 1],
                                            in0=ps, scalar1=Sc("p_b0", 6 * h + j),
                                            scalar2=0.0, op0=ADD, op1=MAX)
                for j in range(KG):
                    ps = pac1.tile([128, 1], f32, tag="a1")
                    for k in range(PJ):
                        nc.tensor.matmul(ps, next_block(),
                                         ph0T[:, 6 * h + k:6 * h + k + 1],
                                         start=(k == 0), stop=(k == PJ - 1))
                    nc.vector.tensor_copy(out=p_part[:, 48 * h + j:48 * h + j + 1],
                                          in_=ps)

            # p-head collective (overlaps with card stream below)
            pin = dpool.tile([128, 144], f32)
            pout = dpool.tile([128, 144], f32)
            nc.sync.dma_start(out=pin, in_=p_part)
            nc.gpsimd.collective_compute(
                "AllReduce", ADD, replica_groups=[list(range(NCORES))],
                ins=[pin.opt()], outs=[pout.opt()])
            nc.sync.dma_start(out=p_red, in_=pout)
            nc.vector.tensor_add(ph1, p_red, S("p_b1"))
            nc.vector.tensor_scalar_max(out=ph1, in0=ph1, scalar1=0.0)

            # ---------------- card head -------------------------------------
            for j in range(CJ):
                psg = pac1.tile([128, 1], f32, tag="a1")
                psh = pac10.tile([128, 20], f32, tag="emb")
                for k in range(KG):
                    nc.tensor.matmul(psg, next_block(), gn[:, k:k + 1],
                                     start=(k == 0), stop=(k == KG - 1))
                for k in range(KH):
                    nc.tensor.matmul(psh[:, :10], next_block(),
                                     xhT_w[:, k * 10:(k + 1) * 10],
                                     start=(k == 0), stop=(k == KH - 1))
                tmpb = temps.tile([128, 1], f32, tag="tmpb")
                nc.vector.tensor_add(tmpb, psg, Sc("card_b0T", j))
                nc.vector.tensor_scalar(out=h0cT[:, j * 10:(j + 1) * 10],
                                        in0=psh[:, :10], scalar1=tmpb,
                                        scalar2=0.0, op0=ADD, op1=MAX)
            for j in range(52):
                ps = pac10.tile([128, 20], f32, tag="emb")
                for k in range(CK2):
                    nc.tensor.matmul(ps[:, :10], next_block(),
                                     h0cT[:, k * 10:(k + 1) * 10],
                                     start=(k == 0), stop=(k == CK2 - 1))
                nc.vector.tensor_copy(out=card_part[:, j * 10:(j + 1) * 10],
                                      in_=ps[:, :10])
            assert state["b"] == NB, state["b"]

            # p-head Wout (fills PE while card collective runs)
            psov = pers.tile([1, 3], f32)
            for h in range(3):
                pw = pmisc.tile([1, 1], f32, tag="m1", name="pw%d" % h)
                for k in range(KG):
                    nc.tensor.matmul(pw, Sc("pWoutT", 48 * h + k),
                                     ph1[:, 48 * h + k:48 * h + k + 1],
                                     start=(k == 0), stop=(k == KG - 1))
                nc.vector.tensor_copy(out=psov[:, h:h + 1], in_=pw)

            # card collective
            cin = dpool.tile([128, 520], f32)
            cout = dpool.tile([128, 520], f32)
            nc.sync.dma_start(out=cin, in_=card_part)
            nc.gpsimd.collective_compute(
                "AllReduce", ADD, replica_groups=[list(range(NCORES))],
                ins=[cin.opt()], outs=[cout.opt()])
            nc.sync.dma_start(out=c_red, in_=cout)
            nc.vector.tensor_add(ch1, c_red, S("card_b1bc"))
            nc.vector.tensor_scalar_max(out=ch1, in0=ch1, scalar1=0.0)

            # card Wout: two [1,10] psums (verifier rejects partition-1 reads)
            pco0 = pmisc.tile([1, 10], f32, tag="pco0")
            pco1 = pmisc.tile([1, 10], f32, tag="pco1")
            for k in range(52):
                co = Sc("cardWoutT", k, w=2)
                sl = ch1[:, k * 10:(k + 1) * 10]
                nc.tensor.matmul(pco0, co[:, 0:1], sl,
                                 start=(k == 0), stop=(k == 51))
                nc.tensor.matmul(pco1, co[:, 1:2], sl,
                                 start=(k == 0), stop=(k == 51))

            # ---------------- finale ----------------------------------------
            nc.vector.tensor_copy(out=logit[:, 0:10], in_=pco0)
            nc.vector.tensor_copy(out=logit[:, 10:20], in_=pco1)
            nc.vector.tensor_copy(out=logit[:, 20:22], in_=psov[:, 0:2])
            nc.vector.tensor_add(lg2, logit, S("addvec", 1))
            mx = temps.tile([1, 2], f32, tag="mx")
            nc.vector.reduce_max(out=mx[:, 0:1], in_=lg2, axis=AX)
            nc.vector.tensor_scalar_mul(out=mx[:, 1:2], in0=mx[:, 0:1], scalar1=-1.0)
            nc.scalar.activation(out=ex, in_=lg2,
                                 func=mybir.ActivationFunctionType.Exp,
                                 bias=mx[:, 1:2], scale=1.0)
            ssum = temps.tile([1, 2], f32, tag="ssum")
            nc.vector.reduce_sum(out=ssum[:, 0:1], in_=ex, axis=AX)
            nc.vector.reciprocal(out=ssum[:, 1:2], in_=ssum[:, 0:1])
            nc.vector.tensor_scalar_mul(out=out_sb[:, 0:22], in0=ex,
                                        scalar1=ssum[:, 1:2])
            nc.vector.tensor_add(out_sb[:, 22:23], psov[:, 2:3], S("cbout", 1))
            nc.sync.dma_start(out=out_ext, in_=out_sb)

    nc.compile()
    return nc


# ---------------------------------------------------------------- entry point
_CACHE = {}


def kernel(hand, draw_pile, disc_pile, character, monster, energy,
           valid_action_mask, params):
    from concourse.bass_utils import run_bass_kernel_spmd

    hand = np.asarray(hand, F32)
    draw_pile = np.asarray(draw_pile, F32)
    disc_pile = np.asarray(disc_pile, F32)
    character = np.asarray(character, F32)
    monster = np.asarray(monster, F32)
    energy = np.asarray(energy, F32)
    valid_action_mask = np.asarray(valid_action_mask)
    p = {k: np.asarray(v, F32) for k, v in params.items()}

    in_maps = []
    for c in range(NCORES):
        in_maps.append({
            "wstream": pack_wstream(c, p),
            "smalls": pack_smalls(c, hand, draw_pile, disc_pile, character,
                                  monster, energy, valid_action_mask, p),
        })

    if "nc" not in _CACHE:
        _CACHE["nc"] = build_graph()
    res = run_bass_kernel_spmd(_CACHE["nc"], in_maps, list(range(NCORES)))
    out = np.asarray(res.results[0]["out"], F32).reshape(23)
    return out[:22].copy(), out[22:23].copy()


if __name__ == "__main__":
    pass
